# revision 1
# baseline (speedup 1.0000x reference)
"""MHA kernel for TRN2, data-parallel over batch across 8 NeuronCores.

Problem (hardcoded shapes):
  x [128, 256, 256] f32 -> leaky_relu -> @W_enc[256,512]+b_enc -> h [128,256,512]
  per head n(8): Q=h[:, :64]@WQ[n], K=h@WK[n], V=h@WV[n]
  scores = Q@K^T/sqrt(512); p = softmax; z = p@V; out = mean_n z  -> [128, 64, 512]

Per-core layout (16 batches = 4096 tokens):
  hT  [128, 4, 4096]  : h transposed (H on partitions, 4 tiles of 128)
  haT [128, 4, 1024]  : agent columns of hT (e<64), contiguous per batch
  per head: qT [128,4,1024]; per batch-pair (512 tokens): kT [128,4,512],
  V natural [128,4,512]; scores/softmax packed 2 batches in 128 partitions.
All matmuls run as float32r (fp32 bits, full-rate PE at N>=256).
"""
import numpy as np
from contextlib import ExitStack

import concourse.bass as bass
from concourse import bacc
import concourse.tile as tile
import concourse.mybir as mybir
from concourse import bass_utils
from concourse.masks import make_identity

F32 = mybir.dt.float32
F32R = mybir.dt.float32r
AF = mybir.ActivationFunctionType

B, E, DIN, H, NH, A = 128, 256, 256, 512, 8, 64
NCORES = 8
BC = B // NCORES        # batches per core
TOK = BC * E            # tokens per core
NTB = TOK // 512        # encode token blocks
NBP = BC // 2           # batch pairs
SCALE = float(1.0 / np.sqrt(H))




def build():
    nc = bacc.Bacc(name="mha_dp")
    x_d = nc.dram_tensor("x", [TOK, DIN], F32, kind="ExternalInput")
    wenc_d = nc.dram_tensor("w_enc", [DIN, H], F32R, kind="ExternalInput")
    benc_d = nc.dram_tensor("b_enc", [H], F32, kind="ExternalInput")
    wq_d = nc.dram_tensor("wq", [NH, H, H], F32R, kind="ExternalInput")
    wk_d = nc.dram_tensor("wk", [NH, H, H], F32R, kind="ExternalInput")
    wv_d = nc.dram_tensor("wv", [NH, H, H], F32R, kind="ExternalInput")
    out_d = nc.dram_tensor("out", [BC * A, H], F32, kind="ExternalOutput")

    with ExitStack() as ctx:
        tc = ctx.enter_context(tile.TileContext(nc))
        const = ctx.enter_context(tc.tile_pool(name="const", bufs=1))
        big = ctx.enter_context(tc.tile_pool(name="big", bufs=1))

        ident = const.tile([128, 128], F32)
        make_identity(nc, ident[:])
        wenc = const.tile([128, 2, H], F32R)
        nc.sync.dma_start(wenc[:], wenc_d.rearrange("(k p) h -> p k h", p=128))
        bias = const.tile([128, 4], F32)
        nc.sync.dma_start(bias[:], benc_d.rearrange("(m p) -> p m", p=128))

        hT = big.tile([128, 4, TOK], F32R)
        haT = big.tile([128, 4, BC * A], F32R)
        out_acc = big.tile([128, NBP, H], F32)

        # ---------------- encode ----------------
        with ExitStack() as ectx:
            epool = ectx.enter_context(tc.tile_pool(name="enc", bufs=3))
            epsum = ectx.enter_context(tc.tile_pool(name="encps", bufs=2, space="PSUM"))
            for tb in range(NTB):
                xin = epool.tile([128, 4, DIN], F32, tag="xin")
                nc.sync.dma_start(
                    xin[:],
                    x_d[tb * 512:(tb + 1) * 512].rearrange("(s p) d -> p s d", p=128),
                )
                xl = epool.tile([128, 4, DIN], F32, tag="xl")
                nc.scalar.activation(xl[:], xin[:], AF.Lrelu, alpha=0.01)
                xt = epool.tile([128, 2, 512], F32R, tag="xt")
                for kt in range(2):
                    pst = epsum.tile([128, 512], F32, tag="pst")
                    for s in range(4):
                        nc.tensor.transpose(
                            pst[:, s * 128:(s + 1) * 128],
                            xl[:, s, kt * 128:(kt + 1) * 128],
                            ident[:],
                        )
                    nc.vector.tensor_copy(xt[:, kt, :], pst[:])
                for m in range(4):
                    ph = epsum.tile([128, 512], F32, tag="ph")
                    for kt in range(2):
                        nc.tensor.matmul(
                            ph[:],
                            wenc[:, kt, m * 128:(m + 1) * 128],
                            xt[:, kt, :],
                            start=(kt == 0),
                            stop=(kt == 1),
                        )
                    nc.vector.tensor_scalar_add(
                        hT[:, m, tb * 512:(tb + 1) * 512], ph[:], bias[:, m:m + 1]
                    )
                    # agent columns (e<64 of each of the 2 batches in this block)
                    nc.vector.tensor_copy(
                        haT[:, m, tb * 128:(tb + 1) * 128],
                        ph.rearrange("p (c e) -> p c e", e=256)[:, :, 0:A],
                    )

        # ---------------- heads ----------------
        wpool = ctx.enter_context(tc.tile_pool(name="w", bufs=2))
        qpool = ctx.enter_context(tc.tile_pool(name="qp", bufs=1))
        hpool = ctx.enter_context(tc.tile_pool(name="hp", bufs=2))
        sfx = ctx.enter_context(tc.tile_pool(name="sfx", bufs=2))
        ps_kv = ctx.enter_context(tc.tile_pool(name="pskv", bufs=4, space="PSUM"))
        ps_s = ctx.enter_context(tc.tile_pool(name="pss", bufs=2, space="PSUM"))
        ps_z = ctx.enter_context(tc.tile_pool(name="psz", bufs=2, space="PSUM"))

        for n in range(NH):
            wq = wpool.tile([128, 4, H], F32R, tag="wq")
            wk = wpool.tile([128, 4, H], F32R, tag="wk")
            wv = wpool.tile([128, 4, H], F32R, tag="wv")
            nc.sync.dma_start(wq[:], wq_d[n].rearrange("(k p) d -> p k d", p=128))
            nc.sync.dma_start(wk[:], wk_d[n].rearrange("(k p) d -> p k d", p=128))
            nc.sync.dma_start(wv[:], wv_d[n].rearrange("(k p) d -> p k d", p=128))

            qT = qpool.tile([128, 4, BC * A], F32R, tag="qT")
            for m in range(4):
                for hf in range(2):
                    pq = ps_kv.tile([128, 512], F32, tag="kv")
                    for kt in range(4):
                        nc.tensor.matmul(
                            pq[:],
                            wq[:, kt, m * 128:(m + 1) * 128],
                            haT[:, kt, hf * 512:(hf + 1) * 512],
                            start=(kt == 0),
                            stop=(kt == 3),
                        )
                    nc.vector.tensor_copy(qT[:, m, hf * 512:(hf + 1) * 512], pq[:])

            for bp in range(NBP):
                t0 = bp * 512
                kT = hpool.tile([128, 4, 512], F32R, tag="kT")
                for m in range(4):
                    pk = ps_kv.tile([128, 512], F32, tag="kv")
                    for kt in range(4):
                        nc.tensor.matmul(
                            pk[:],
                            wk[:, kt, m * 128:(m + 1) * 128],
                            hT[:, kt, t0:t0 + 512],
                            start=(kt == 0),
                            stop=(kt == 3),
                        )
                    nc.vector.tensor_copy(kT[:, m, :], pk[:])
                vN = hpool.tile([128, 4, H], F32R, tag="vN")
                for tt in range(4):
                    pv = ps_kv.tile([128, 512], F32, tag="kv")
                    for kt in range(4):
                        nc.tensor.matmul(
                            pv[:],
                            hT[:, kt, t0 + tt * 128:t0 + (tt + 1) * 128],
                            wv[:, kt, :],
                            start=(kt == 0),
                            stop=(kt == 3),
                        )
                    nc.vector.tensor_copy(vN[:, tt, :], pv[:])

                # scores: one M=64 matmul chain per batch, packed to 128
                # partitions in SBUF for the softmax
                sin = sfx.tile([128, 256], F32, tag="sin")
                for c in range(2):
                    b = 2 * bp + c
                    ps = ps_s.tile([128, 256], F32, tag="s256")
                    for m in range(4):
                        nc.tensor.matmul(
                            ps[0:64, :],
                            qT[:, m, b * A:(b + 1) * A],
                            kT[:, m, c * 256:(c + 1) * 256],
                            start=(m == 0),
                            stop=(m == 3),
                        )
                    nc.vector.tensor_copy(sin[c * 64:(c + 1) * 64, :], ps[0:64, :])
                # softmax over free dim (entities)
                rmax = sfx.tile([128, 1], F32, tag="rmax")
                nc.vector.reduce_max(rmax[:], sin[:], axis=mybir.AxisListType.X)
                nb = sfx.tile([128, 1], F32, tag="nb")
                nc.vector.tensor_scalar_mul(nb[:], rmax[:], -SCALE)
                pex = sfx.tile([128, 256], F32, tag="pex")
                rsum = sfx.tile([128, 1], F32, tag="rsum")
                nc.scalar.activation(
                    pex[:], sin[:], AF.Exp, bias=nb[:], scale=SCALE, accum_out=rsum[:]
                )
                rcp = sfx.tile([128, 1], F32, tag="rcp")
                nc.vector.reciprocal(rcp[:], rsum[:])
                pn = sfx.tile([128, 256], F32, tag="pn")
                nc.vector.tensor_scalar_mul(pn[:], pex[:], rcp[:])
                # transpose p -> [e, packed agents]
                pt_ps = ps_s.tile([128, 256], F32, tag="s256")
                for ke in range(2):
                    nc.tensor.transpose(
                        pt_ps[:, ke * 128:(ke + 1) * 128],
                        pn[:, ke * 128:(ke + 1) * 128],
                        ident[:],
                    )
                pt = sfx.tile([128, 256], F32R, tag="ptsb")
                nc.vector.tensor_copy(pt[:], pt_ps[:])
                # z = p @ V, one M=64 chain per batch
                for c in range(2):
                    pz = ps_z.tile([128, H], F32, tag="z")
                    for ke in range(2):
                        nc.tensor.matmul(
                            pz[0:64, :],
                            pt[:, ke * 128 + c * 64:ke * 128 + (c + 1) * 64],
                            vN[:, 2 * c + ke, :],
                            start=(ke == 0),
                            stop=(ke == 1),
                        )
                    dst = out_acc[c * 64:(c + 1) * 64, bp, :]
                    if n == 0:
                        nc.vector.tensor_copy(dst, pz[0:64, :])
                    else:
                        nc.vector.tensor_tensor(
                            dst, dst, pz[0:64, :], op=mybir.AluOpType.add,
                        )

        for bp in range(NBP):
            nc.vector.tensor_scalar_mul(
                out_acc[:, bp, :], out_acc[:, bp, :], 1.0 / NH
            )
        nc.sync.dma_start(out_d.rearrange("(t p) d -> p t d", p=128), out_acc[:])
    nc.finalize()
    return nc


_NC_CACHE = None


def kernel(x, W_enc, b_enc, WQ, WK, WV, n_agents=None, **_unused):
    global _NC_CACHE
    x = np.ascontiguousarray(np.asarray(x, dtype=np.float32))
    W_enc = np.ascontiguousarray(np.asarray(W_enc, dtype=np.float32))
    b_enc = np.ascontiguousarray(np.asarray(b_enc, dtype=np.float32))
    WQ = np.ascontiguousarray(np.asarray(WQ, dtype=np.float32))
    WK = np.ascontiguousarray(np.asarray(WK, dtype=np.float32))
    WV = np.ascontiguousarray(np.asarray(WV, dtype=np.float32))

    if _NC_CACHE is None:
        _NC_CACHE = build()
    nc = _NC_CACHE

    in_maps = []
    for ci in range(NCORES):
        xs = x[ci * BC:(ci + 1) * BC].reshape(TOK, DIN)
        in_maps.append({
            "x": np.ascontiguousarray(xs),
            "w_enc": W_enc, "b_enc": b_enc,
            "wq": WQ, "wk": WK, "wv": WV,
        })
    res = bass_utils.run_bass_kernel_spmd(nc, in_maps, core_ids=list(range(NCORES)))
    out = np.empty((B, A, H), dtype=np.float32)
    for ci in range(NCORES):
        out[ci * BC:(ci + 1) * BC] = res.results[ci]["out"].reshape(BC, A, H)
    return out



# revision 2
# speedup vs baseline: 3.9975x; 3.9975x over previous
"""MHA kernel for TRN2, data-parallel over batch across 8 NeuronCores.

Problem (hardcoded shapes):
  x [128, 256, 256] f32 -> leaky_relu -> @W_enc[256,512]+b_enc -> h [128,256,512]
  per head n(8): Q=h[:, :64]@WQ[n], K=h@WK[n], V=h@WV[n]
  scores = Q@K^T/sqrt(512); p = softmax; z = p@V; out = mean_n z  -> [128, 64, 512]

Host<->device traffic is the bottleneck (axon tunnel ~45MB/s), so the wire
format is fp16 and the weights are sharded across cores and AllGathered
on-device over NeuronLink instead of being replicated through the tunnel:
  core i ships x[16i:16(i+1)] (fp16), WQ[i];WK[i];WV[i] stacked [1536,512]
  (fp16) and W_enc rows [32i,32(i+1)) (fp16). Device compute stays f32r.

Per-core layout (16 batches = 4096 tokens):
  hT  [128, 4, 4096]  : h transposed (H on partitions, 4 tiles of 128)
  haT [128, 4, 1024]  : agent columns of hT (e<64), contiguous per batch
  per head: qT [128,4,1024]; per batch-pair (512 tokens): kT [128,4,512],
  V natural [128,4,512]; scores/softmax packed 2 batches in 128 partitions.
All matmuls run as float32r (fp32 bits, full-rate PE at N>=256).
"""
import numpy as np
from contextlib import ExitStack

import concourse.bass as bass
from concourse import bacc
import concourse.tile as tile
import concourse.mybir as mybir
from concourse import bass_utils
from concourse.masks import make_identity

F32 = mybir.dt.float32
F32R = mybir.dt.float32r
F16 = mybir.dt.float16
AF = mybir.ActivationFunctionType

B, E, DIN, H, NH, A = 128, 256, 256, 512, 8, 64
NCORES = 8
BC = B // NCORES        # batches per core
TOK = BC * E            # tokens per core
NTB = TOK // 512        # encode token blocks
NBP = BC // 2           # batch pairs
SCALE = float(1.0 / np.sqrt(H))
WROWS = 3 * H           # rows of this core's stacked WQ|WK|WV shard
ER = DIN // NCORES      # W_enc rows per core


def build():
    nc = bacc.Bacc(name="mha_dp_ag")
    x_d = nc.dram_tensor("x", [TOK, DIN], F16, kind="ExternalInput")
    wencsh_d = nc.dram_tensor("wencsh", [ER, H], F16, kind="ExternalInput")
    benc_d = nc.dram_tensor("b_enc", [H], F32, kind="ExternalInput")
    wsh_d = nc.dram_tensor("wsh", [WROWS, H], F16, kind="ExternalInput")
    out_d = nc.dram_tensor("out", [BC * A, H], F16, kind="ExternalOutput")

    with ExitStack() as ctx:
        tc = ctx.enter_context(tile.TileContext(nc))
        dram = ctx.enter_context(tc.tile_pool(name="dram", bufs=1, space="DRAM"))
        const = ctx.enter_context(tc.tile_pool(name="const", bufs=1))
        big = ctx.enter_context(tc.tile_pool(name="big", bufs=1))

        # ---- AllGather the sharded weights over NeuronLink ----
        wsh_b = dram.tile([WROWS, H], F16)
        wall_b = dram.tile([NCORES * WROWS, H], F16)
        nc.gpsimd.dma_start(wsh_b[:], wsh_d[:])
        nc.gpsimd.collective_compute(
            "AllGather", mybir.AluOpType.bypass,
            replica_groups=[list(range(NCORES))],
            ins=[wsh_b.opt()], outs=[wall_b.opt()],
        )
        wencsh_b = dram.tile([ER, H], F16)
        wenc_b = dram.tile([DIN, H], F16)
        nc.gpsimd.dma_start(wencsh_b[:], wencsh_d[:])
        nc.gpsimd.collective_compute(
            "AllGather", mybir.AluOpType.bypass,
            replica_groups=[list(range(NCORES))],
            ins=[wencsh_b.opt()], outs=[wenc_b.opt()],
        )

        ident = const.tile([128, 128], F32)
        make_identity(nc, ident[:])
        wenc16 = const.tile([128, 2, H], F16)
        nc.sync.dma_start(wenc16[:], wenc_b.rearrange("(k p) h -> p k h", p=128))
        wenc = const.tile([128, 2, H], F32R)
        nc.vector.tensor_copy(wenc[:], wenc16[:])
        bias = const.tile([128, 4], F32)
        nc.sync.dma_start(bias[:], benc_d.rearrange("(m p) -> p m", p=128))

        hT = big.tile([128, 4, TOK], F32R)
        haT = big.tile([128, 4, BC * A], F32R)
        out_acc = big.tile([128, NBP, H], F32)

        # ---------------- encode ----------------
        with ExitStack() as ectx:
            epool = ectx.enter_context(tc.tile_pool(name="enc", bufs=3))
            epsum = ectx.enter_context(tc.tile_pool(name="encps", bufs=2, space="PSUM"))
            for tb in range(NTB):
                xin = epool.tile([128, 4, DIN], F16, tag="xin")
                nc.sync.dma_start(
                    xin[:],
                    x_d[tb * 512:(tb + 1) * 512].rearrange("(s p) d -> p s d", p=128),
                )
                xl = epool.tile([128, 4, DIN], F32, tag="xl")
                nc.scalar.activation(xl[:], xin[:], AF.Lrelu, alpha=0.01)
                xt = epool.tile([128, 2, 512], F32R, tag="xt")
                for kt in range(2):
                    pst = epsum.tile([128, 512], F32, tag="pst")
                    for s in range(4):
                        nc.tensor.transpose(
                            pst[:, s * 128:(s + 1) * 128],
                            xl[:, s, kt * 128:(kt + 1) * 128],
                            ident[:],
                        )
                    nc.vector.tensor_copy(xt[:, kt, :], pst[:])
                for m in range(4):
                    ph = epsum.tile([128, 512], F32, tag="ph")
                    for kt in range(2):
                        nc.tensor.matmul(
                            ph[:],
                            wenc[:, kt, m * 128:(m + 1) * 128],
                            xt[:, kt, :],
                            start=(kt == 0),
                            stop=(kt == 1),
                        )
                    nc.vector.tensor_scalar_add(
                        hT[:, m, tb * 512:(tb + 1) * 512], ph[:], bias[:, m:m + 1]
                    )
                    # agent columns (e<64 of each of the 2 batches in this block)
                    nc.vector.tensor_copy(
                        haT[:, m, tb * 128:(tb + 1) * 128],
                        ph.rearrange("p (c e) -> p c e", e=256)[:, :, 0:A],
                    )

        # ---------------- heads ----------------
        wpool = ctx.enter_context(tc.tile_pool(name="w", bufs=1))
        qpool = ctx.enter_context(tc.tile_pool(name="qp", bufs=1))
        hpool = ctx.enter_context(tc.tile_pool(name="hp", bufs=2))
        sfx = ctx.enter_context(tc.tile_pool(name="sfx", bufs=2))
        ps_kv = ctx.enter_context(tc.tile_pool(name="pskv", bufs=4, space="PSUM"))
        ps_s = ctx.enter_context(tc.tile_pool(name="pss", bufs=2, space="PSUM"))
        ps_z = ctx.enter_context(tc.tile_pool(name="psz", bufs=2, space="PSUM"))

        for n in range(NH):
            w16 = wpool.tile([128, 3, 4, H], F16, tag="w16")
            r0 = n * WROWS
            for m in range(3):
                nc.sync.dma_start(
                    w16[:, m],
                    wall_b[r0 + m * H:r0 + (m + 1) * H].rearrange(
                        "(k p) d -> p k d", p=128
                    ),
                )
            wq = wpool.tile([128, 4, H], F32R, tag="wq")
            wk = wpool.tile([128, 4, H], F32R, tag="wk")
            wv = wpool.tile([128, 4, H], F32R, tag="wv")
            nc.vector.tensor_copy(wq[:], w16[:, 0])
            nc.vector.tensor_copy(wk[:], w16[:, 1])
            nc.vector.tensor_copy(wv[:], w16[:, 2])

            qT = qpool.tile([128, 4, BC * A], F32R, tag="qT")
            for m in range(4):
                for hf in range(2):
                    pq = ps_kv.tile([128, 512], F32, tag="kv")
                    for kt in range(4):
                        nc.tensor.matmul(
                            pq[:],
                            wq[:, kt, m * 128:(m + 1) * 128],
                            haT[:, kt, hf * 512:(hf + 1) * 512],
                            start=(kt == 0),
                            stop=(kt == 3),
                        )
                    nc.vector.tensor_copy(qT[:, m, hf * 512:(hf + 1) * 512], pq[:])

            for bp in range(NBP):
                t0 = bp * 512
                kT = hpool.tile([128, 4, 512], F32R, tag="kT")
                for m in range(4):
                    pk = ps_kv.tile([128, 512], F32, tag="kv")
                    for kt in range(4):
                        nc.tensor.matmul(
                            pk[:],
                            wk[:, kt, m * 128:(m + 1) * 128],
                            hT[:, kt, t0:t0 + 512],
                            start=(kt == 0),
                            stop=(kt == 3),
                        )
                    nc.vector.tensor_copy(kT[:, m, :], pk[:])
                vN = hpool.tile([128, 4, H], F32R, tag="vN")
                for tt in range(4):
                    pv = ps_kv.tile([128, 512], F32, tag="kv")
                    for kt in range(4):
                        nc.tensor.matmul(
                            pv[:],
                            hT[:, kt, t0 + tt * 128:t0 + (tt + 1) * 128],
                            wv[:, kt, :],
                            start=(kt == 0),
                            stop=(kt == 3),
                        )
                    nc.vector.tensor_copy(vN[:, tt, :], pv[:])

                # scores: one M=64 matmul chain per batch, packed to 128
                # partitions in SBUF for the softmax
                sin = sfx.tile([128, 256], F32, tag="sin")
                for c in range(2):
                    b = 2 * bp + c
                    ps = ps_s.tile([128, 256], F32, tag="s256")
                    for m in range(4):
                        nc.tensor.matmul(
                            ps[0:64, :],
                            qT[:, m, b * A:(b + 1) * A],
                            kT[:, m, c * 256:(c + 1) * 256],
                            start=(m == 0),
                            stop=(m == 3),
                        )
                    nc.vector.tensor_copy(sin[c * 64:(c + 1) * 64, :], ps[0:64, :])
                # softmax over free dim (entities)
                rmax = sfx.tile([128, 1], F32, tag="rmax")
                nc.vector.reduce_max(rmax[:], sin[:], axis=mybir.AxisListType.X)
                nb = sfx.tile([128, 1], F32, tag="nb")
                nc.vector.tensor_scalar_mul(nb[:], rmax[:], -SCALE)
                pex = sfx.tile([128, 256], F32, tag="pex")
                rsum = sfx.tile([128, 1], F32, tag="rsum")
                nc.scalar.activation(
                    pex[:], sin[:], AF.Exp, bias=nb[:], scale=SCALE, accum_out=rsum[:]
                )
                rcp = sfx.tile([128, 1], F32, tag="rcp")
                nc.vector.reciprocal(rcp[:], rsum[:])
                pn = sfx.tile([128, 256], F32, tag="pn")
                nc.vector.tensor_scalar_mul(pn[:], pex[:], rcp[:])
                # transpose p -> [e, packed agents]
                pt_ps = ps_s.tile([128, 256], F32, tag="s256")
                for ke in range(2):
                    nc.tensor.transpose(
                        pt_ps[:, ke * 128:(ke + 1) * 128],
                        pn[:, ke * 128:(ke + 1) * 128],
                        ident[:],
                    )
                pt = sfx.tile([128, 256], F32R, tag="ptsb")
                nc.vector.tensor_copy(pt[:], pt_ps[:])
                # z = p @ V, one M=64 chain per batch
                for c in range(2):
                    pz = ps_z.tile([128, H], F32, tag="z")
                    for ke in range(2):
                        nc.tensor.matmul(
                            pz[0:64, :],
                            pt[:, ke * 128 + c * 64:ke * 128 + (c + 1) * 64],
                            vN[:, 2 * c + ke, :],
                            start=(ke == 0),
                            stop=(ke == 1),
                        )
                    dst = out_acc[c * 64:(c + 1) * 64, bp, :]
                    if n == 0:
                        nc.vector.tensor_copy(dst, pz[0:64, :])
                    else:
                        nc.vector.tensor_tensor(
                            dst, dst, pz[0:64, :], op=mybir.AluOpType.add,
                        )

        opool = ctx.enter_context(tc.tile_pool(name="o16", bufs=2))
        for bp in range(NBP):
            o16 = opool.tile([128, H], F16, tag="o16")
            nc.vector.tensor_scalar_mul(o16[:], out_acc[:, bp, :], 1.0 / NH)
            nc.sync.dma_start(out_d[bp * 128:(bp + 1) * 128], o16[:])
    nc.finalize()
    return nc


_NC_CACHE = None


def kernel(x, W_enc, b_enc, WQ, WK, WV, n_agents=None, **_unused):
    global _NC_CACHE
    x16 = np.asarray(x, dtype=np.float16)
    wenc16 = np.asarray(W_enc, dtype=np.float16)
    b_enc = np.ascontiguousarray(np.asarray(b_enc, dtype=np.float32))
    wq16 = np.asarray(WQ, dtype=np.float16)
    wk16 = np.asarray(WK, dtype=np.float16)
    wv16 = np.asarray(WV, dtype=np.float16)

    if _NC_CACHE is None:
        _NC_CACHE = build()
    nc = _NC_CACHE

    in_maps = []
    for ci in range(NCORES):
        in_maps.append({
            "x": np.ascontiguousarray(
                x16[ci * BC:(ci + 1) * BC].reshape(TOK, DIN)
            ),
            "wencsh": np.ascontiguousarray(wenc16[ci * ER:(ci + 1) * ER]),
            "b_enc": b_enc,
            "wsh": np.concatenate([wq16[ci], wk16[ci], wv16[ci]], axis=0),
        })
    res = bass_utils.run_bass_kernel_spmd(nc, in_maps, core_ids=list(range(NCORES)))
    out = np.empty((B, A, H), dtype=np.float32)
    for ci in range(NCORES):
        out[ci * BC:(ci + 1) * BC] = res.results[ci]["out"].reshape(BC, A, H)
    return out


# revision 4
# speedup vs baseline: 6.7974x; 1.7004x over previous
"""MHA kernel for TRN2, data-parallel over batch across 8 NeuronCores.

Problem (hardcoded shapes):
  x [128, 256, 256] f32 -> leaky_relu -> @W_enc[256,512]+b_enc -> h [128,256,512]
  per head n(8): Q=h[:, :64]@WQ[n], K=h@WK[n], V=h@WV[n]
  scores = Q@K^T/sqrt(512); p = softmax; z = p@V; out = mean_n z  -> [128, 64, 512]

Host<->device traffic is the bottleneck (axon tunnel ~45MB/s), so the wire
format is fp16 and the weights are sharded across cores and AllGathered
on-device over NeuronLink instead of being replicated through the tunnel:
  core i ships x[16i:16(i+1)] (fp16), WQ[i];WK[i];WV[i] stacked [1536,512]
  (fp16) and W_enc rows [32i,32(i+1)) (fp16). Device compute stays f32r.

Per-core layout (16 batches = 4096 tokens):
  hT  [128, 4, 4096]  : h transposed (H on partitions, 4 tiles of 128)
  haT [128, 4, 1024]  : agent columns of hT (e<64), contiguous per batch
  per head: qT [128,4,1024]; per batch-pair (512 tokens): kT [128,4,512],
  V natural [128,4,512]; scores/softmax packed 2 batches in 128 partitions.
All matmuls run as float32r (fp32 bits, full-rate PE at N>=256).
"""
import numpy as np
from concurrent.futures import ThreadPoolExecutor
from contextlib import ExitStack

import jax

# The per-call jax.jit re-trace inside run_bass_kernel_spmd recompiles the
# XLA wrapper every call (~0.35s); the persistent cache makes that a hit.
try:
    jax.config.update("jax_compilation_cache_dir", "/root/.jax_cache")
    jax.config.update("jax_persistent_cache_min_entry_size_bytes", -1)
    jax.config.update("jax_persistent_cache_min_compile_time_secs", 0)
except Exception:
    pass

import concourse.bass as bass
from concourse import bacc
import concourse.tile as tile
import concourse.mybir as mybir
from concourse import bass_utils
from concourse.masks import make_identity

F32 = mybir.dt.float32
F32R = mybir.dt.float32r
F16 = mybir.dt.float16
AF = mybir.ActivationFunctionType

B, E, DIN, H, NH, A = 128, 256, 256, 512, 8, 64
NCORES = 8
BC = B // NCORES        # batches per core
TOK = BC * E            # tokens per core
NTB = TOK // 512        # encode token blocks
NBP = BC // 2           # batch pairs
SCALE = float(1.0 / np.sqrt(H))
WROWS = 3 * H           # rows of this core's stacked WQ|WK|WV shard
ER = DIN // NCORES      # W_enc rows per core


def build():
    nc = bacc.Bacc(name="mha_dp_ag")
    x_d = nc.dram_tensor("x", [TOK, DIN], F16, kind="ExternalInput")
    wencsh_d = nc.dram_tensor("wencsh", [ER, H], F16, kind="ExternalInput")
    benc_d = nc.dram_tensor("b_enc", [H], F32, kind="ExternalInput")
    wsh_d = nc.dram_tensor("wsh", [WROWS, H], F16, kind="ExternalInput")
    out_d = nc.dram_tensor("out", [BC * A, H], F16, kind="ExternalOutput")

    with ExitStack() as ctx:
        tc = ctx.enter_context(tile.TileContext(nc))
        dram = ctx.enter_context(tc.tile_pool(name="dram", bufs=1, space="DRAM"))
        const = ctx.enter_context(tc.tile_pool(name="const", bufs=1))
        big = ctx.enter_context(tc.tile_pool(name="big", bufs=1))

        # ---- AllGather the sharded weights over NeuronLink ----
        wsh_b = dram.tile([WROWS, H], F16)
        wall_b = dram.tile([NCORES * WROWS, H], F16)
        nc.gpsimd.dma_start(wsh_b[:], wsh_d[:])
        nc.gpsimd.collective_compute(
            "AllGather", mybir.AluOpType.bypass,
            replica_groups=[list(range(NCORES))],
            ins=[wsh_b.opt()], outs=[wall_b.opt()],
        )
        wencsh_b = dram.tile([ER, H], F16)
        wenc_b = dram.tile([DIN, H], F16)
        nc.gpsimd.dma_start(wencsh_b[:], wencsh_d[:])
        nc.gpsimd.collective_compute(
            "AllGather", mybir.AluOpType.bypass,
            replica_groups=[list(range(NCORES))],
            ins=[wencsh_b.opt()], outs=[wenc_b.opt()],
        )

        ident = const.tile([128, 128], F32)
        make_identity(nc, ident[:])
        wenc16 = const.tile([128, 2, H], F16)
        nc.sync.dma_start(wenc16[:], wenc_b.rearrange("(k p) h -> p k h", p=128))
        wenc = const.tile([128, 2, H], F32R)
        nc.vector.tensor_copy(wenc[:], wenc16[:])
        bias = const.tile([128, 4], F32)
        nc.sync.dma_start(bias[:], benc_d.rearrange("(m p) -> p m", p=128))

        hT = big.tile([128, 4, TOK], F32R)
        haT = big.tile([128, 4, BC * A], F32R)
        out_acc = big.tile([128, NBP, H], F32)

        # ---------------- encode ----------------
        with ExitStack() as ectx:
            epool = ectx.enter_context(tc.tile_pool(name="enc", bufs=3))
            epsum = ectx.enter_context(tc.tile_pool(name="encps", bufs=2, space="PSUM"))
            for tb in range(NTB):
                xin = epool.tile([128, 4, DIN], F16, tag="xin")
                nc.sync.dma_start(
                    xin[:],
                    x_d[tb * 512:(tb + 1) * 512].rearrange("(s p) d -> p s d", p=128),
                )
                xl = epool.tile([128, 4, DIN], F32, tag="xl")
                nc.scalar.activation(xl[:], xin[:], AF.Lrelu, alpha=0.01)
                xt = epool.tile([128, 2, 512], F32R, tag="xt")
                for kt in range(2):
                    pst = epsum.tile([128, 512], F32, tag="pst")
                    for s in range(4):
                        nc.tensor.transpose(
                            pst[:, s * 128:(s + 1) * 128],
                            xl[:, s, kt * 128:(kt + 1) * 128],
                            ident[:],
                        )
                    nc.vector.tensor_copy(xt[:, kt, :], pst[:])
                for m in range(4):
                    ph = epsum.tile([128, 512], F32, tag="ph")
                    for kt in range(2):
                        nc.tensor.matmul(
                            ph[:],
                            wenc[:, kt, m * 128:(m + 1) * 128],
                            xt[:, kt, :],
                            start=(kt == 0),
                            stop=(kt == 1),
                        )
                    nc.vector.tensor_scalar_add(
                        hT[:, m, tb * 512:(tb + 1) * 512], ph[:], bias[:, m:m + 1]
                    )
                    # agent columns (e<64 of each of the 2 batches in this block)
                    nc.vector.tensor_copy(
                        haT[:, m, tb * 128:(tb + 1) * 128],
                        ph.rearrange("p (c e) -> p c e", e=256)[:, :, 0:A],
                    )

        # ---------------- heads ----------------
        wpool = ctx.enter_context(tc.tile_pool(name="w", bufs=1))
        qpool = ctx.enter_context(tc.tile_pool(name="qp", bufs=1))
        hpool = ctx.enter_context(tc.tile_pool(name="hp", bufs=2))
        sfx = ctx.enter_context(tc.tile_pool(name="sfx", bufs=2))
        ps_kv = ctx.enter_context(tc.tile_pool(name="pskv", bufs=4, space="PSUM"))
        ps_s = ctx.enter_context(tc.tile_pool(name="pss", bufs=2, space="PSUM"))
        ps_z = ctx.enter_context(tc.tile_pool(name="psz", bufs=2, space="PSUM"))

        for n in range(NH):
            w16 = wpool.tile([128, 3, 4, H], F16, tag="w16")
            r0 = n * WROWS
            for m in range(3):
                nc.sync.dma_start(
                    w16[:, m],
                    wall_b[r0 + m * H:r0 + (m + 1) * H].rearrange(
                        "(k p) d -> p k d", p=128
                    ),
                )
            wq = wpool.tile([128, 4, H], F32R, tag="wq")
            wk = wpool.tile([128, 4, H], F32R, tag="wk")
            wv = wpool.tile([128, 4, H], F32R, tag="wv")
            nc.vector.tensor_copy(wq[:], w16[:, 0])
            nc.vector.tensor_copy(wk[:], w16[:, 1])
            nc.vector.tensor_copy(wv[:], w16[:, 2])

            qT = qpool.tile([128, 4, BC * A], F32R, tag="qT")
            for m in range(4):
                for hf in range(2):
                    pq = ps_kv.tile([128, 512], F32, tag="kv")
                    for kt in range(4):
                        nc.tensor.matmul(
                            pq[:],
                            wq[:, kt, m * 128:(m + 1) * 128],
                            haT[:, kt, hf * 512:(hf + 1) * 512],
                            start=(kt == 0),
                            stop=(kt == 3),
                        )
                    nc.vector.tensor_copy(qT[:, m, hf * 512:(hf + 1) * 512], pq[:])

            for bp in range(NBP):
                t0 = bp * 512
                kT = hpool.tile([128, 4, 512], F32R, tag="kT")
                for m in range(4):
                    pk = ps_kv.tile([128, 512], F32, tag="kv")
                    for kt in range(4):
                        nc.tensor.matmul(
                            pk[:],
                            wk[:, kt, m * 128:(m + 1) * 128],
                            hT[:, kt, t0:t0 + 512],
                            start=(kt == 0),
                            stop=(kt == 3),
                        )
                    nc.vector.tensor_copy(kT[:, m, :], pk[:])
                vN = hpool.tile([128, 4, H], F32R, tag="vN")
                for tt in range(4):
                    pv = ps_kv.tile([128, 512], F32, tag="kv")
                    for kt in range(4):
                        nc.tensor.matmul(
                            pv[:],
                            hT[:, kt, t0 + tt * 128:t0 + (tt + 1) * 128],
                            wv[:, kt, :],
                            start=(kt == 0),
                            stop=(kt == 3),
                        )
                    nc.vector.tensor_copy(vN[:, tt, :], pv[:])

                # scores: one M=64 matmul chain per batch, packed to 128
                # partitions in SBUF for the softmax
                sin = sfx.tile([128, 256], F32, tag="sin")
                for c in range(2):
                    b = 2 * bp + c
                    ps = ps_s.tile([128, 256], F32, tag="s256")
                    for m in range(4):
                        nc.tensor.matmul(
                            ps[0:64, :],
                            qT[:, m, b * A:(b + 1) * A],
                            kT[:, m, c * 256:(c + 1) * 256],
                            start=(m == 0),
                            stop=(m == 3),
                        )
                    nc.vector.tensor_copy(sin[c * 64:(c + 1) * 64, :], ps[0:64, :])
                # softmax over free dim (entities)
                rmax = sfx.tile([128, 1], F32, tag="rmax")
                nc.vector.reduce_max(rmax[:], sin[:], axis=mybir.AxisListType.X)
                nb = sfx.tile([128, 1], F32, tag="nb")
                nc.vector.tensor_scalar_mul(nb[:], rmax[:], -SCALE)
                pex = sfx.tile([128, 256], F32, tag="pex")
                rsum = sfx.tile([128, 1], F32, tag="rsum")
                nc.scalar.activation(
                    pex[:], sin[:], AF.Exp, bias=nb[:], scale=SCALE, accum_out=rsum[:]
                )
                rcp = sfx.tile([128, 1], F32, tag="rcp")
                nc.vector.reciprocal(rcp[:], rsum[:])
                pn = sfx.tile([128, 256], F32, tag="pn")
                nc.vector.tensor_scalar_mul(pn[:], pex[:], rcp[:])
                # transpose p -> [e, packed agents]
                pt_ps = ps_s.tile([128, 256], F32, tag="s256")
                for ke in range(2):
                    nc.tensor.transpose(
                        pt_ps[:, ke * 128:(ke + 1) * 128],
                        pn[:, ke * 128:(ke + 1) * 128],
                        ident[:],
                    )
                pt = sfx.tile([128, 256], F32R, tag="ptsb")
                nc.vector.tensor_copy(pt[:], pt_ps[:])
                # z = p @ V, one M=64 chain per batch
                for c in range(2):
                    pz = ps_z.tile([128, H], F32, tag="z")
                    for ke in range(2):
                        nc.tensor.matmul(
                            pz[0:64, :],
                            pt[:, ke * 128 + c * 64:ke * 128 + (c + 1) * 64],
                            vN[:, 2 * c + ke, :],
                            start=(ke == 0),
                            stop=(ke == 1),
                        )
                    dst = out_acc[c * 64:(c + 1) * 64, bp, :]
                    if n == 0:
                        nc.vector.tensor_copy(dst, pz[0:64, :])
                    else:
                        nc.vector.tensor_tensor(
                            dst, dst, pz[0:64, :], op=mybir.AluOpType.add,
                        )

        opool = ctx.enter_context(tc.tile_pool(name="o16", bufs=2))
        for bp in range(NBP):
            o16 = opool.tile([128, H], F16, tag="o16")
            nc.vector.tensor_scalar_mul(o16[:], out_acc[:, bp, :], 1.0 / NH)
            nc.sync.dma_start(out_d[bp * 128:(bp + 1) * 128], o16[:])
    nc.finalize()
    return nc


_NC_CACHE = None


def kernel(x, W_enc, b_enc, WQ, WK, WV, n_agents=None, **_unused):
    global _NC_CACHE
    with ThreadPoolExecutor(max_workers=5) as ex:
        fx = ex.submit(lambda: np.asarray(x, dtype=np.float16))
        fwe = ex.submit(lambda: np.asarray(W_enc, dtype=np.float16))
        fwq = ex.submit(lambda: np.asarray(WQ, dtype=np.float16))
        fwk = ex.submit(lambda: np.asarray(WK, dtype=np.float16))
        fwv = ex.submit(lambda: np.asarray(WV, dtype=np.float16))
        x16, wenc16 = fx.result(), fwe.result()
        wq16, wk16, wv16 = fwq.result(), fwk.result(), fwv.result()
    b_enc = np.ascontiguousarray(np.asarray(b_enc, dtype=np.float32))

    if _NC_CACHE is None:
        _NC_CACHE = build()
    nc = _NC_CACHE

    in_maps = []
    for ci in range(NCORES):
        in_maps.append({
            "x": np.ascontiguousarray(
                x16[ci * BC:(ci + 1) * BC].reshape(TOK, DIN)
            ),
            "wencsh": np.ascontiguousarray(wenc16[ci * ER:(ci + 1) * ER]),
            "b_enc": b_enc,
            "wsh": np.concatenate([wq16[ci], wk16[ci], wv16[ci]], axis=0),
        })
    res = bass_utils.run_bass_kernel_spmd(nc, in_maps, core_ids=list(range(NCORES)))
    out = np.empty((B, A, H), dtype=np.float32)
    for ci in range(NCORES):
        out[ci * BC:(ci + 1) * BC] = res.results[ci]["out"].reshape(BC, A, H)
    return out
